# revision 1
# baseline (speedup 1.0000x reference)
"""LSTM regression kernel for 8 Trainium2 NeuronCores.

Model (reference): B=2048, IN=2048, H=1024, T=15 steps, x constant across
steps. Data-parallel over batch: each of the 8 cores handles 256 batch rows.

Device strategy (per core, batch BL=256):
 - Everything kept "transposed": state hT/cT stored as [H, BL] with H on
   partitions (8 chunks of 128), so no per-step transposes are needed.
 - gatesT[4H, BL] = W_hh @ hT accumulated in PSUM over 8 K-chunks, plus one
   extra identity-weight matmul that adds the precomputed xgT tile (this
   replaces a per-tile DVE add of the input-gate contribution).
 - xgT[4H, BL] = W_ihAug @ xAugT computed once at start; biases b_ih+b_hh are
   folded in host-side by augmenting x with a ones-row and W_ih with a bias
   row (padded to a whole 128-row chunk).
 - Activations (sigmoid/tanh) on ScalarE directly from PSUM; cell update on
   VectorE per 128-row h-chunk so it pipelines with the matmuls.
 - Matmul inputs in fp16 (fp32 PSUM accumulate) - all operands here are
   small-range, so fp16's 10-bit mantissa beats bf16 at identical PE speed.
   h kept in fp32 for output and re-cast to fp16 each step.
"""

import os
import numpy as np
import ml_dtypes

try:
    import concourse.bass as bass
except ImportError:  # pragma: no cover
    import sys
    sys.path.insert(0, "/opt/trn_rl_repo")
    import concourse.bass as bass
from concourse import bacc
import concourse.mybir as mybir
import concourse.tile as tile
from concourse.bass_utils import run_bass_kernel_spmd
from concourse.masks import make_identity

F32 = mybir.dt.float32
F16 = mybir.dt.float16
AF = mybir.ActivationFunctionType

T = 15
B, IN, H = 2048, 2048, 1024
NCORES = 8
BL = B // NCORES            # 256 batch rows per core
G4 = 4 * H                  # 4096 gate rows
NM = G4 // 128              # 32 gate m-tiles
NKH = H // 128              # 8 hidden K-chunks
INA = IN + 128              # x augmented with ones row, padded to chunk
NKX = INA // 128            # 17 input K-chunks
INIT = 0.01

LAST_EXEC_NS = None
LAST_RESULTS = None

_cached_nc = None


def _build():
    nc = bacc.Bacc(None, target_bir_lowering=False)
    wih_hi = nc.dram_tensor("wih_hi", [INA, G4], F16, kind="ExternalInput")
    wih_lo = nc.dram_tensor("wih_lo", [INA, G4], F16, kind="ExternalInput")
    whh = nc.dram_tensor("whh", [H, G4], F16, kind="ExternalInput")
    xt_hi = nc.dram_tensor("xt_hi", [INA, BL], F16, kind="ExternalInput")
    xt_lo = nc.dram_tensor("xt_lo", [INA, BL], F16, kind="ExternalInput")
    hs = nc.dram_tensor("hs", [T, 128, NKH * BL], F32, kind="ExternalOutput")

    with tile.TileContext(nc) as tc:
        with (
            tc.tile_pool(name="const", bufs=1) as constp,
            tc.tile_pool(name="wihp", bufs=4) as wihp,
            tc.tile_pool(name="state", bufs=2) as statep,
            tc.tile_pool(name="gates", bufs=3) as gatesp,
            tc.tile_pool(name="psum", bufs=8, space="PSUM") as psump,
        ):
            whh_sb = constp.tile([128, NKH * G4], F16, tag="whh")
            xg_hi = constp.tile([128, NM * BL], F16, tag="xghi")
            xg_lo = constp.tile([128, NM * BL], F16, tag="xglo")
            xth_sb = constp.tile([128, NKX * BL], F16, tag="xth")
            xtl_sb = constp.tile([128, NKX * BL], F16, tag="xtl")
            ident = constp.tile([128, 128], F16, tag="ident")
            make_identity(nc, ident[:, :])

            whh_r = whh[:, :].rearrange("(kc p) m -> kc p m", p=128)
            for kc in range(NKH):
                nc.sync.dma_start(whh_sb[:, kc * G4:(kc + 1) * G4], whh_r[kc])
            xth_r = xt_hi[:, :].rearrange("(kc p) b -> kc p b", p=128)
            xtl_r = xt_lo[:, :].rearrange("(kc p) b -> kc p b", p=128)
            for kc in range(NKX):
                nc.sync.dma_start(xth_sb[:, kc * BL:(kc + 1) * BL], xth_r[kc])
                nc.sync.dma_start(xtl_sb[:, kc * BL:(kc + 1) * BL], xtl_r[kc])

            # ---- xg phase: 4 sweeps, each producing ALL 4 gates for an
            # hc-pair (so recurrent step 0 for hc 0..1 can start after the
            # first sweep and overlap the rest of the xg phase) ----
            for sweep in range(4):
                pstiles = [psump.tile([128, BL], F32, tag="ps", name=f"psxg{i}") for i in range(8)]
                for kc in range(NKX):
                    wth = wihp.tile([128, 1024], F16, tag="wihh", name="wth")
                    wtl = wihp.tile([128, 1024], F16, tag="wihl", name="wtl")
                    src_h = wih_hi[kc * 128:(kc + 1) * 128, :].rearrange(
                        "p (g t c) -> p g t c", g=4, t=4
                    )[:, :, sweep, :]
                    src_l = wih_lo[kc * 128:(kc + 1) * 128, :].rearrange(
                        "p (g t c) -> p g t c", g=4, t=4
                    )[:, :, sweep, :]
                    nc.sync.dma_start(wth[:, :], src_h)
                    nc.sync.dma_start(wtl[:, :], src_l)
                    for ml in range(8):
                        for pi, (wt_, xt_) in enumerate(
                            [(wth, xth_sb), (wth, xtl_sb), (wtl, xth_sb)]
                        ):
                            nc.tensor.matmul(
                                pstiles[ml][:, :],
                                wt_[:, ml * 128:(ml + 1) * 128],
                                xt_[:, kc * BL:(kc + 1) * BL],
                                start=(kc == 0 and pi == 0),
                                stop=(kc == NKX - 1 and pi == 2),
                            )
                for ml in range(8):
                    g_, j_ = ml // 2, ml % 2
                    m = g_ * 8 + sweep * 2 + j_
                    nc.scalar.copy(xg_hi[:, m * BL:(m + 1) * BL], pstiles[ml][:, :])
                    nc.vector.tensor_sub(
                        xg_lo[:, m * BL:(m + 1) * BL],
                        pstiles[ml][:, :],
                        xg_hi[:, m * BL:(m + 1) * BL],
                    )

            # ---- recurrent steps ----
            h_prev = statep.tile([128, NKH * BL], F16, tag="hbf")
            c_prev = statep.tile([128, NKH * BL], F32, tag="c")
            nc.any.memset(h_prev[:, :], INIT)
            nc.any.memset(c_prev[:, :], INIT)

            for t in range(T):
                h_bf = statep.tile([128, NKH * BL], F16, tag="hbf")
                h_f32 = statep.tile([128, NKH * BL], F32, tag="hf")
                c_new = statep.tile([128, NKH * BL], F32, tag="c")
                for hc in range(NKH):
                    gt = []
                    for gi in range(4):
                        m = gi * NKH + hc
                        ps = psump.tile([128, BL], F32, tag="ps", name="psrec")
                        for kc in range(NKH):
                            nc.tensor.matmul(
                                ps[:, :],
                                whh_sb[:, kc * G4 + m * 128: kc * G4 + (m + 1) * 128],
                                h_prev[:, kc * BL:(kc + 1) * BL],
                                start=(kc == 0),
                                stop=False,
                            )
                        nc.tensor.matmul(
                            ps[:, :],
                            ident[:, :],
                            xg_hi[:, m * BL:(m + 1) * BL],
                            start=False,
                            stop=False,
                        )
                        nc.tensor.matmul(
                            ps[:, :],
                            ident[:, :],
                            xg_lo[:, m * BL:(m + 1) * BL],
                            start=False,
                            stop=True,
                        )
                        g = gatesp.tile([128, BL], F32, tag=f"g{gi}", name=f"gate{gi}")
                        fn = AF.Tanh if gi == 2 else AF.Sigmoid
                        nc.scalar.activation(g[:, :], ps[:, :], fn)
                        gt.append(g)
                    sl = slice(hc * BL, (hc + 1) * BL)
                    t0 = gatesp.tile([128, BL], F32, tag="t0")
                    t1 = gatesp.tile([128, BL], F32, tag="t1")
                    th = gatesp.tile([128, BL], F32, tag="th")
                    nc.vector.tensor_mul(t0[:, :], gt[0][:, :], gt[2][:, :])
                    nc.vector.tensor_mul(t1[:, :], gt[1][:, :], c_prev[:, sl])
                    nc.vector.tensor_add(c_new[:, sl], t0[:, :], t1[:, :])
                    nc.scalar.activation(th[:, :], c_new[:, sl], AF.Tanh)
                    nc.vector.tensor_mul(h_f32[:, sl], gt[3][:, :], th[:, :])
                    nc.vector.tensor_copy(h_bf[:, sl], h_f32[:, sl])
                nc.sync.dma_start(hs[t], h_f32[:, :])
                h_prev, c_prev = h_bf, c_new

    nc.compile()
    return nc


def timeline_ns():
    from concourse.timeline_sim import TimelineSim
    nc = _get_nc()
    ts = TimelineSim(nc)
    ts.simulate()
    return ts.time


def _get_nc():
    global _cached_nc
    if _cached_nc is None:
        _cached_nc = _build()
    return _cached_nc


def kernel(x, W_ih, W_hh, b_ih, b_hh):
    global LAST_EXEC_NS, LAST_RESULTS
    nc = _get_nc()
    bf = np.float16
    x = np.asarray(x, np.float32)
    W_ih = np.asarray(W_ih, np.float32)
    W_hh = np.asarray(W_hh, np.float32)
    b_ih = np.asarray(b_ih, np.float32)
    b_hh = np.asarray(b_hh, np.float32)

    def hilo(a):
        hi = a.astype(bf)
        lo = (a - hi.astype(np.float32)).astype(bf)
        return hi, lo

    waug = np.zeros((INA, G4), np.float32)
    waug[:IN] = W_ih.T
    waug[IN] = b_ih + b_hh
    waug_hi, waug_lo = hilo(waug)
    whh_bf = np.ascontiguousarray(W_hh.T).astype(bf)

    in_maps = []
    for c in range(NCORES):
        xa = np.zeros((INA, BL), np.float32)
        xa[:IN] = x[c * BL:(c + 1) * BL].T
        xa[IN] = 1.0
        xa_hi, xa_lo = hilo(xa)
        in_maps.append({
            "wih_hi": waug_hi, "wih_lo": waug_lo, "whh": whh_bf,
            "xt_hi": xa_hi, "xt_lo": xa_lo,
        })

    trace = os.environ.get("LSTM_TRACE") == "1"
    res = run_bass_kernel_spmd(
        nc, in_maps, core_ids=list(range(NCORES)), trace=trace
    )
    LAST_EXEC_NS = res.exec_time_ns
    LAST_RESULTS = res

    out = np.empty((T, B, H), np.float32)
    for c in range(NCORES):
        a = res.results[c]["hs"].reshape(T, 128, NKH, BL)
        out[:, c * BL:(c + 1) * BL, :] = (
            a.transpose(0, 3, 2, 1).reshape(T, BL, H)
        )
    return out



# revision 16
# speedup vs baseline: 1.5501x; 1.5501x over previous
"""LSTM regression kernel for 8 Trainium2 NeuronCores.

Model (reference): B=2048, IN=2048, H=1024, T=15 steps, x constant across
steps. Data-parallel over batch: each of the 8 cores handles 256 batch rows.

Per-core design (BL=256 batch cols, everything kept transposed [dim, BL]):
 - xg[4H, BL] = W_aug.T @ x_aug computed once (x augmented with a ones row
   that carries b_ih+b_hh), fp16 inputs / f32 PSUM, stored f32 in SBUF.
 - Step 0 is free of matmuls: h0=c0=0.01 const, so W_hh@h0 = 0.01*rowsum(W_hh)
   is folded into the activation bias (per-partition [128,1] bias per m-tile).
 - Steps 1..14: gates = xg + W_hh @ h_t. The W_hh matmul accumulates in PSUM
   (8 K-chunks of 128); the xg add happens on VectorE (PSUM+SBUF->SBUF), NOT
   as identity matmuls - keeps the PE stream pure W_hh work.
 - Gate quarters are ordered [i, f, o, g] per h-chunk so ScalarE can run one
   sigmoid over 768 cols + one tanh over 256 cols.
 - Cell update on VectorE; i,g,o,tanh(c) in fp16 (2x DVE mode), c stays f32.
 - h stored fp16 (feeds next step's matmul and the output DMA; host upcasts).
"""

import os
import numpy as np

try:
    import concourse.bass as bass
except ImportError:  # pragma: no cover
    import sys
    sys.path.insert(0, "/opt/trn_rl_repo")
    import concourse.bass as bass
from concourse import bacc
import concourse.mybir as mybir
import concourse.tile as tile
from concourse.bass_utils import run_bass_kernel_spmd

F32 = mybir.dt.float32
F16 = mybir.dt.float16
AF = mybir.ActivationFunctionType

T = 15
B, IN, H = 2048, 2048, 1024
NCORES = 8
BL = B // NCORES            # 256 batch rows per core
G4 = 4 * H                  # 4096 gate rows
NM = G4 // 128              # 32 gate m-tiles
NKH = H // 128              # 8 hidden K-chunks
INA = IN + 128              # x augmented with ones row, padded to chunk
NKX = INA // 128            # 17 input K-chunks
NHC = NKH                   # 8 h-output chunks
INIT = 0.01
QGATE = (0, 1, 3, 2)        # quarter -> gate index (i, f, o, g)

LAST_EXEC_NS = None
LAST_RESULTS = None

_cached_nc = None


def _build():
    nc = bacc.Bacc(None, target_bir_lowering=False)
    wih = nc.dram_tensor("wih", [NHC, NKX, 128, 4 * 128], F16, kind="ExternalInput")
    whh = nc.dram_tensor("whh", [NKH, 128, G4], F16, kind="ExternalInput")
    xp = nc.dram_tensor("xp", [NKX, 128, BL], F16, kind="ExternalInput")
    bias0 = nc.dram_tensor("bias0", [128, NM], F32, kind="ExternalInput")
    hs = nc.dram_tensor("hs", [T, 128, NKH * BL], F16, kind="ExternalOutput")

    with tile.TileContext(nc) as tc:
        with (
            tc.tile_pool(name="const", bufs=1) as constp,
            tc.tile_pool(name="wihp", bufs=3) as wpool,
            tc.tile_pool(name="state", bufs=3) as statep,
            tc.tile_pool(name="gates", bufs=4) as gp,
            tc.tile_pool(name="psum", bufs=8, space="PSUM") as psump,
        ):
            whh_sb = constp.tile([128, NKH * G4], F16, tag="whh")
            xg_sb = constp.tile([128, NM * BL], F16, tag="xg")
            x_sb = constp.tile([128, NKX * BL], F16, tag="xsb")
            bias_sb = constp.tile([128, NM], F32, tag="bias")

            # x: first K-chunk alone so the first matmul can start ASAP
            nc.sync.dma_start(x_sb[:, 0:BL], xp[0])
            nc.sync.dma_start(
                x_sb[:, BL:].rearrange("p (kc b) -> p kc b", kc=NKX - 1),
                xp[1:, :, :].rearrange("kc p b -> p kc b"),
            )
            nc.sync.dma_start(bias_sb[:, :], bias0[:, :])

            def cell_update(hc, ifo, g16, c_prev, c_new, h16):
                sl = slice(hc * BL, (hc + 1) * BL)
                t0 = gp.tile([128, BL], F16, tag="t0")
                t1 = gp.tile([128, BL], F32, tag="t1")
                th = gp.tile([128, BL], F16, tag="th")
                nc.vector.tensor_mul(t0[:, :], ifo[:, 0:BL], g16[:, :])
                nc.vector.tensor_mul(t1[:, :], ifo[:, BL:2 * BL], c_prev[:, sl])
                nc.vector.tensor_add(c_new[:, sl], t0[:, :], t1[:, :])
                nc.scalar.activation(th[:, :], c_new[:, sl], AF.Tanh)
                nc.vector.tensor_mul(h16[:, sl], ifo[:, 2 * BL:3 * BL], th[:, :])

            # ---- xg phase + step 0 (no matmuls for the recurrent part) ----
            c_prev = statep.tile([128, NKH * BL], F32, tag="c")
            nc.vector.memset(c_prev[:, :], INIT)
            h16 = statep.tile([128, NKH * BL], F16, tag="h16")
            c_new = statep.tile([128, NKH * BL], F32, tag="c")

            for hc in range(NHC):
                wt = wpool.tile([128, NKX * 512], F16, tag="wt")
                if hc == 0:
                    # split the first tile's DMA so kc=0 lands quickly
                    for a, b in ((0, 2), (2, 6), (6, 11), (11, NKX)):
                        nc.sync.dma_start(
                            wt[:, a * 512:b * 512].rearrange(
                                "p (kc c) -> p kc c", kc=b - a
                            ),
                            wih[hc, a:b].rearrange("kc p c -> p kc c"),
                        )
                else:
                    nc.sync.dma_start(
                        wt[:, :].rearrange("p (kc c) -> p kc c", kc=NKX),
                        wih[hc].rearrange("kc p c -> p kc c"),
                    )
                ifo = gp.tile([128, 3 * BL], F16, tag="ifo")
                g16 = gp.tile([128, BL], F16, tag="g16")
                for q in range(4):
                    m_abs = QGATE[q] * NKH + hc
                    ps = psump.tile([128, BL], F32, tag="ps")
                    for kc in range(NKX):
                        nc.tensor.matmul(
                            ps[:, :],
                            wt[:, (kc * 4 + q) * 128:(kc * 4 + q + 1) * 128],
                            x_sb[:, kc * BL:(kc + 1) * BL],
                            start=(kc == 0),
                            stop=(kc == NKX - 1),
                        )
                    fn = AF.Tanh if q == 3 else AF.Sigmoid
                    dst = g16[:, :] if q == 3 else ifo[:, q * BL:(q + 1) * BL]
                    nc.scalar.activation(
                        dst, ps[:, :], fn, bias=bias_sb[:, m_abs:m_abs + 1]
                    )
                    nc.vector.tensor_copy(
                        xg_sb[:, (hc * 4 + q) * BL:(hc * 4 + q + 1) * BL], ps[:, :]
                    )
                cell_update(hc, ifo, g16, c_prev, c_new, h16)
                if hc >= 4:
                    # interleave W_hh chunk loads into the tail of the W_ih
                    # stream (W_hh is only needed from step 1)
                    kc = hc - 4
                    nc.sync.dma_start(whh_sb[:, kc * G4:(kc + 1) * G4], whh[kc])
            nc.sync.dma_start(hs[0], h16[:, :])
            h_prev, c_prev = h16, c_new

            for kc in range(4, NKH):
                nc.sync.dma_start(whh_sb[:, kc * G4:(kc + 1) * G4], whh[kc])

            # ---- recurrent steps 1..14 ----
            for t in range(1, T):
                h16 = statep.tile([128, NKH * BL], F16, tag="h16")
                c_new = statep.tile([128, NKH * BL], F32, tag="c")
                for hc in range(NHC):
                    g32 = gp.tile([128, 4 * BL], F32, tag="g32")
                    ifo = gp.tile([128, 3 * BL], F16, tag="ifo")
                    g16 = gp.tile([128, BL], F16, tag="g16")
                    pstiles = {}
                    # defer the kc=7 matmuls of hc0's quarters so the PE has
                    # runway while the previous step's last h-chunk finishes
                    defer = 1 if hc == 0 else 0
                    last = hc == NHC - 1
                    # last chunk: quarters in (i, g, f, o) order + per-piece
                    # activations so the h16 chain closes sooner
                    qseq = (0, 3, 1, 2) if last else (0, 1, 2, 3)
                    for q in qseq:
                        m_abs = QGATE[q] * NKH + hc
                        ps = psump.tile([128, BL], F32, tag="ps")
                        pstiles[q] = ps
                        for kc in range(NKH - defer):
                            nc.tensor.matmul(
                                ps[:, :],
                                whh_sb[:, kc * G4 + m_abs * 128:
                                       kc * G4 + (m_abs + 1) * 128],
                                h_prev[:, kc * BL:(kc + 1) * BL],
                                start=(kc == 0),
                                stop=(kc == NKH - 1),
                            )
                    if defer:
                        kc = NKH - 1
                        for q in qseq:
                            m_abs = QGATE[q] * NKH + hc
                            nc.tensor.matmul(
                                pstiles[q][:, :],
                                whh_sb[:, kc * G4 + m_abs * 128:
                                       kc * G4 + (m_abs + 1) * 128],
                                h_prev[:, kc * BL:(kc + 1) * BL],
                                start=False,
                                stop=True,
                            )

                    def add_q(q):
                        nc.vector.tensor_add(
                            g32[:, q * BL:(q + 1) * BL],
                            pstiles[q][:, :],
                            xg_sb[:, (hc * 4 + q) * BL:(hc * 4 + q + 1) * BL],
                        )

                    if last:
                        for q in qseq:
                            add_q(q)
                            fn = AF.Tanh if q == 3 else AF.Sigmoid
                            dst = (g16[:, :] if q == 3
                                   else ifo[:, q * BL:(q + 1) * BL])
                            nc.scalar.activation(dst, g32[:, q * BL:(q + 1) * BL], fn)
                    else:
                        for q in range(4):
                            add_q(q)
                        nc.scalar.activation(ifo[:, :], g32[:, 0:3 * BL], AF.Sigmoid)
                        nc.scalar.activation(g16[:, :], g32[:, 3 * BL:4 * BL], AF.Tanh)
                    cell_update(hc, ifo, g16, c_prev, c_new, h16)
                    if t == T - 1:
                        # last step: stream out each h-chunk as it completes
                        # so the kernel tail isn't one big dependent DMA
                        nc.sync.dma_start(
                            hs[t, :, hc * BL:(hc + 1) * BL],
                            h16[:, hc * BL:(hc + 1) * BL],
                        )
                if t < T - 1:
                    nc.sync.dma_start(hs[t], h16[:, :])
                h_prev, c_prev = h16, c_new

    nc.compile()
    return nc


def timeline_ns():
    from concourse.timeline_sim import TimelineSim
    nc = _get_nc()
    ts = TimelineSim(nc)
    ts.simulate()
    return ts.time


def _get_nc():
    global _cached_nc
    if _cached_nc is None:
        _cached_nc = _build()
    return _cached_nc


def kernel(x, W_ih, W_hh, b_ih, b_hh):
    global LAST_EXEC_NS, LAST_RESULTS
    nc = _get_nc()
    x = np.asarray(x, np.float32)
    W_ih = np.asarray(W_ih, np.float32)
    W_hh = np.asarray(W_hh, np.float32)
    b_ih = np.asarray(b_ih, np.float32)
    b_hh = np.asarray(b_hh, np.float32)

    waug = np.zeros((INA, G4), np.float32)
    waug[:IN] = W_ih.T
    waug[IN] = b_ih + b_hh
    # [kc][p][gi][hc][c] -> quarter order (i,f,o,g) -> [hc][kc][p][q*128+c]
    wih_pack = np.ascontiguousarray(
        waug.reshape(NKX, 128, 4, NKH, 128)[:, :, QGATE, :, :]
        .transpose(3, 0, 1, 2, 4)
        .reshape(NHC, NKX, 128, 512)
    ).astype(np.float16)
    whh_pack = np.ascontiguousarray(W_hh.T.reshape(NKH, 128, G4)).astype(np.float16)
    bias_pack = np.ascontiguousarray(
        (INIT * W_hh.sum(1)).reshape(NM, 128).T
    ).astype(np.float32)

    in_maps = []
    for c in range(NCORES):
        xa = np.zeros((INA, BL), np.float32)
        xa[:IN] = x[c * BL:(c + 1) * BL].T
        xa[IN] = 1.0
        in_maps.append({
            "wih": wih_pack,
            "whh": whh_pack,
            "xp": xa.reshape(NKX, 128, BL).astype(np.float16),
            "bias0": bias_pack,
        })

    trace = os.environ.get("LSTM_TRACE") == "1"
    res = run_bass_kernel_spmd(
        nc, in_maps, core_ids=list(range(NCORES)), trace=trace
    )
    LAST_EXEC_NS = res.exec_time_ns
    LAST_RESULTS = res

    out = np.empty((T, B, H), np.float32)
    for c in range(NCORES):
        a = np.asarray(res.results[c]["hs"], np.float32).reshape(T, 128, NKH, BL)
        out[:, c * BL:(c + 1) * BL, :] = a.transpose(0, 3, 2, 1).reshape(T, BL, H)
    return out


# revision 24
# speedup vs baseline: 1.6140x; 1.0412x over previous
"""LSTM regression kernel for 8 Trainium2 NeuronCores.

Model (reference): B=2048, IN=2048, H=1024, T=15 steps, x constant across
steps. Data-parallel over batch: each of the 8 cores handles 256 batch rows.

Per-core design (BL=256 batch cols, everything kept transposed [dim, BL]):
 - xg[4H, BL] = W_aug.T @ x_aug computed once (x augmented with a ones row
   that carries b_ih+b_hh), fp16 inputs / f32 PSUM, stored fp16 in SBUF.
 - Step 0 is free of matmuls: h0=c0=0.01 const, so W_hh@h0 = 0.01*rowsum(W_hh)
   is folded into the activation bias (per-partition [128,1] bias per m-tile).
 - Steps 1..14: gates = xg + W_hh @ h_t. The W_hh matmul accumulates in PSUM
   (8 K-chunks of 128, one PSUM bank per gate quarter); the xg add happens on
   VectorE (PSUM+SBUF->SBUF), NOT as identity matmuls - keeps the PE stream
   pure W_hh work (256 N=256 matmuls/step, the fp16 PE roofline).
 - Step boundary: the first two h-chunks' matmuls are staggered (hc0 kc0-6,
   hc1 kc0-6, then both kc7 groups) so the PE has ~6us of runway before it
   reads the previous step's last h-chunks / the last W_hh DMA chunk (t=1).
 - Gate quarters are ordered [i, f, o, g] per h-chunk so ScalarE can run one
   sigmoid over 768 cols + one tanh over 256 cols; the last h-chunk uses
   per-piece activations in (i,g,f,o) order to close its h16 chain sooner.
 - Cell update on VectorE; i,g,o,tanh(c) in fp16 (2x DVE mode), c stays f32.
 - h stored fp16 (feeds next step's matmul and the output DMA; host upcasts).
 - DMA: W_ih streamed as 8 half-MB-scale tile DMAs (first one split for fast
   start), W_hh as 8 chunk DMAs interleaved into the W_ih stream tail; the
   pre-step-1 phase is DMA-bound at ~360GB/s and fully packed.
"""

import os
import numpy as np

try:
    import concourse.bass as bass
except ImportError:  # pragma: no cover
    import sys
    sys.path.insert(0, "/opt/trn_rl_repo")
    import concourse.bass as bass
from concourse import bacc
import concourse.mybir as mybir
import concourse.tile as tile
from concourse.bass_utils import run_bass_kernel_spmd

F32 = mybir.dt.float32
F16 = mybir.dt.float16
AF = mybir.ActivationFunctionType

T = 15
B, IN, H = 2048, 2048, 1024
NCORES = 8
BL = B // NCORES            # 256 batch rows per core
G4 = 4 * H                  # 4096 gate rows
NM = G4 // 128              # 32 gate m-tiles
NKH = H // 128              # 8 hidden K-chunks
INA = IN + 128              # x augmented with ones row, padded to chunk
NKX = INA // 128            # 17 input K-chunks
NHC = NKH                   # 8 h-output chunks
INIT = 0.01
QGATE = (0, 1, 3, 2)        # quarter -> gate index (i, f, o, g)

LAST_EXEC_NS = None
LAST_RESULTS = None

_cached_nc = None


def _build():
    nc = bacc.Bacc(None, target_bir_lowering=False)
    wih = nc.dram_tensor("wih", [NHC, NKX, 128, 4 * 128], F16, kind="ExternalInput")
    whh = nc.dram_tensor("whh", [NKH, 128, G4], F16, kind="ExternalInput")
    xp = nc.dram_tensor("xp", [NKX, 128, BL], F16, kind="ExternalInput")
    bias0 = nc.dram_tensor("bias0", [128, NM], F32, kind="ExternalInput")
    hs = nc.dram_tensor("hs", [T, 128, NKH * BL], F16, kind="ExternalOutput")

    with tile.TileContext(nc) as tc:
        with (
            tc.tile_pool(name="const", bufs=1) as constp,
            tc.tile_pool(name="wihp", bufs=3) as wpool,
            tc.tile_pool(name="state", bufs=2) as statep,
            tc.tile_pool(name="gates", bufs=3) as gp,
            tc.tile_pool(name="psum", bufs=8, space="PSUM") as psump,
        ):
            whh_sb = constp.tile([128, NKH * G4], F16, tag="whh")
            xg_sb = constp.tile([128, NM * BL], F16, tag="xg")
            x_sb = constp.tile([128, NKX * BL], F16, tag="xsb")
            bias_sb = constp.tile([128, NM], F32, tag="bias")

            # x: first K-chunk alone so the first matmul can start ASAP
            nc.sync.dma_start(x_sb[:, 0:BL], xp[0])
            nc.sync.dma_start(
                x_sb[:, BL:].rearrange("p (kc b) -> p kc b", kc=NKX - 1),
                xp[1:, :, :].rearrange("kc p b -> p kc b"),
            )
            nc.sync.dma_start(bias_sb[:, :], bias0[:, :])

            def cell_update(hc, ifo, g16, c_prev, c_new, h16):
                sl = slice(hc * BL, (hc + 1) * BL)
                t0 = gp.tile([128, BL], F16, tag="t0")
                t1 = gp.tile([128, BL], F32, tag="t1")
                th = gp.tile([128, BL], F16, tag="th")
                nc.vector.tensor_mul(t0[:, :], ifo[:, 0:BL], g16[:, :])
                nc.vector.tensor_mul(t1[:, :], ifo[:, BL:2 * BL], c_prev[:, sl])
                nc.vector.tensor_add(c_new[:, sl], t0[:, :], t1[:, :])
                nc.scalar.activation(th[:, :], c_new[:, sl], AF.Tanh)
                nc.vector.tensor_mul(h16[:, sl], ifo[:, 2 * BL:3 * BL], th[:, :])

            # ---- xg phase + step 0 (no matmuls for the recurrent part) ----
            c_prev = statep.tile([128, NKH * BL], F32, tag="c")
            nc.vector.memset(c_prev[:, :], INIT)
            h16 = statep.tile([128, NKH * BL], F16, tag="h16")
            c_new = statep.tile([128, NKH * BL], F32, tag="c")

            for hc in range(NHC):
                wt = wpool.tile([128, NKX * 512], F16, tag="wt")
                if hc == 0:
                    # split the first tile's DMA so kc=0 lands quickly
                    for a, b in ((0, 2), (2, 6), (6, 11), (11, NKX)):
                        nc.sync.dma_start(
                            wt[:, a * 512:b * 512].rearrange(
                                "p (kc c) -> p kc c", kc=b - a
                            ),
                            wih[hc, a:b].rearrange("kc p c -> p kc c"),
                        )
                else:
                    nc.sync.dma_start(
                        wt[:, :].rearrange("p (kc c) -> p kc c", kc=NKX),
                        wih[hc].rearrange("kc p c -> p kc c"),
                    )
                ifo = gp.tile([128, 3 * BL], F16, tag="ifo")
                g16 = gp.tile([128, BL], F16, tag="g16")
                for q in range(4):
                    m_abs = QGATE[q] * NKH + hc
                    ps = psump.tile([128, BL], F32, tag="ps")
                    for kc in range(NKX):
                        nc.tensor.matmul(
                            ps[:, :],
                            wt[:, (kc * 4 + q) * 128:(kc * 4 + q + 1) * 128],
                            x_sb[:, kc * BL:(kc + 1) * BL],
                            start=(kc == 0),
                            stop=(kc == NKX - 1),
                        )
                    fn = AF.Tanh if q == 3 else AF.Sigmoid
                    dst = g16[:, :] if q == 3 else ifo[:, q * BL:(q + 1) * BL]
                    nc.scalar.activation(
                        dst, ps[:, :], fn, bias=bias_sb[:, m_abs:m_abs + 1]
                    )
                    nc.vector.tensor_copy(
                        xg_sb[:, (hc * 4 + q) * BL:(hc * 4 + q + 1) * BL], ps[:, :]
                    )
                cell_update(hc, ifo, g16, c_prev, c_new, h16)
                if hc >= 4:
                    # interleave W_hh chunk loads into the tail of the W_ih
                    # stream (W_hh is only needed from step 1)
                    kc = hc - 4
                    nc.sync.dma_start(whh_sb[:, kc * G4:(kc + 1) * G4], whh[kc])
            for kc in range(4, NKH):
                nc.sync.dma_start(whh_sb[:, kc * G4:(kc + 1) * G4], whh[kc])
            nc.sync.dma_start(hs[0], h16[:, :])
            h_prev, c_prev = h16, c_new

            # ---- recurrent steps 1..14 ----
            def mm_quarter(ps, hc, q, kc, start, stop):
                m_abs = QGATE[q] * NKH + hc
                nc.tensor.matmul(
                    ps[:, :],
                    whh_sb[:, kc * G4 + m_abs * 128: kc * G4 + (m_abs + 1) * 128],
                    h_prev[:, kc * BL:(kc + 1) * BL],
                    start=start,
                    stop=stop,
                )

            def gates_and_cell(hc, pstiles, c_prev, c_new, h16, last):
                g32 = gp.tile([128, 4 * BL], F32, tag="g32")
                ifo = gp.tile([128, 3 * BL], F16, tag="ifo")
                g16 = gp.tile([128, BL], F16, tag="g16")

                def add_q(q):
                    nc.vector.tensor_add(
                        g32[:, q * BL:(q + 1) * BL],
                        pstiles[q][:, :],
                        xg_sb[:, (hc * 4 + q) * BL:(hc * 4 + q + 1) * BL],
                    )

                if last:
                    # per-piece activations in (i, g, f, o) order so the h16
                    # chain closes sooner
                    for q in (0, 3, 1, 2):
                        add_q(q)
                        fn = AF.Tanh if q == 3 else AF.Sigmoid
                        dst = (g16[:, :] if q == 3
                               else ifo[:, q * BL:(q + 1) * BL])
                        nc.scalar.activation(dst, g32[:, q * BL:(q + 1) * BL], fn)
                else:
                    for q in range(4):
                        add_q(q)
                    nc.scalar.activation(ifo[:, :], g32[:, 0:3 * BL], AF.Sigmoid)
                    nc.scalar.activation(g16[:, :], g32[:, 3 * BL:4 * BL], AF.Tanh)
                cell_update(hc, ifo, g16, c_prev, c_new, h16)

            for t in range(1, T):
                h16 = statep.tile([128, NKH * BL], F16, tag="h16")
                c_new = statep.tile([128, NKH * BL], F32, tag="c")
                # head (hc0+hc1): stagger so reads of the previous step's
                # late chunks are pushed out - hc0 kc0-5, hc0 kc6, hc1 kc0-6,
                # then the kc7 reads only after ~6us of runway (also covers
                # the last W_hh DMA chunk still in flight at t=1)
                pst0 = {}
                pst1 = {}
                for q in range(4):
                    ps = psump.tile([128, BL], F32, tag="ps", name=f"ps0_{q}")
                    pst0[q] = ps
                    for kc in range(NKH - 2):
                        mm_quarter(ps, 0, q, kc, start=(kc == 0), stop=False)
                for q in range(4):
                    mm_quarter(pst0[q], 0, q, NKH - 2, start=False, stop=False)
                for q in range(4):
                    ps = psump.tile([128, BL], F32, tag="ps", name=f"ps1_{q}")
                    pst1[q] = ps
                    for kc in range(NKH - 1):
                        mm_quarter(ps, 1, q, kc, start=(kc == 0), stop=False)
                for q in range(4):
                    mm_quarter(pst0[q], 0, q, NKH - 1, start=False, stop=True)
                for q in range(4):
                    mm_quarter(pst1[q], 1, q, NKH - 1, start=False, stop=True)
                gates_and_cell(0, pst0, c_prev, c_new, h16, last=False)
                gates_and_cell(1, pst1, c_prev, c_new, h16, last=False)
                for hc in range(2, NHC):
                    last = hc == NHC - 1
                    pstiles = {}
                    qseq = (0, 3, 1, 2) if last else (0, 1, 2, 3)
                    for q in qseq:
                        ps = psump.tile([128, BL], F32, tag="ps")
                        pstiles[q] = ps
                        for kc in range(NKH):
                            mm_quarter(ps, hc, q, kc,
                                       start=(kc == 0), stop=(kc == NKH - 1))
                    gates_and_cell(hc, pstiles, c_prev, c_new, h16, last)
                    if t == T - 1:
                        # last step: stream out chunks as they complete so the
                        # kernel tail isn't one big dependent DMA
                        if hc == 2:
                            nc.sync.dma_start(hs[t, :, 0:2 * BL], h16[:, 0:2 * BL])
                        if hc >= 2:
                            nc.sync.dma_start(
                                hs[t, :, hc * BL:(hc + 1) * BL],
                                h16[:, hc * BL:(hc + 1) * BL],
                            )
                if t < T - 1:
                    nc.sync.dma_start(hs[t], h16[:, :])
                h_prev, c_prev = h16, c_new

    nc.compile()
    return nc


def timeline_ns():
    from concourse.timeline_sim import TimelineSim
    nc = _get_nc()
    ts = TimelineSim(nc)
    ts.simulate()
    return ts.time


def _get_nc():
    global _cached_nc
    if _cached_nc is None:
        _cached_nc = _build()
    return _cached_nc


def kernel(x, W_ih, W_hh, b_ih, b_hh):
    global LAST_EXEC_NS, LAST_RESULTS
    nc = _get_nc()
    x = np.asarray(x, np.float32)
    W_ih = np.asarray(W_ih, np.float32)
    W_hh = np.asarray(W_hh, np.float32)
    b_ih = np.asarray(b_ih, np.float32)
    b_hh = np.asarray(b_hh, np.float32)

    waug = np.zeros((INA, G4), np.float32)
    waug[:IN] = W_ih.T
    waug[IN] = b_ih + b_hh
    # [kc][p][gi][hc][c] -> quarter order (i,f,o,g) -> [hc][kc][p][q*128+c]
    wih_pack = np.ascontiguousarray(
        waug.reshape(NKX, 128, 4, NKH, 128)[:, :, QGATE, :, :]
        .transpose(3, 0, 1, 2, 4)
        .reshape(NHC, NKX, 128, 512)
    ).astype(np.float16)
    whh_pack = np.ascontiguousarray(W_hh.T.reshape(NKH, 128, G4)).astype(np.float16)
    bias_pack = np.ascontiguousarray(
        (INIT * W_hh.sum(1)).reshape(NM, 128).T
    ).astype(np.float32)

    in_maps = []
    for c in range(NCORES):
        xa = np.zeros((INA, BL), np.float32)
        xa[:IN] = x[c * BL:(c + 1) * BL].T
        xa[IN] = 1.0
        in_maps.append({
            "wih": wih_pack,
            "whh": whh_pack,
            "xp": xa.reshape(NKX, 128, BL).astype(np.float16),
            "bias0": bias_pack,
        })

    trace = os.environ.get("LSTM_TRACE") == "1"
    res = run_bass_kernel_spmd(
        nc, in_maps, core_ids=list(range(NCORES)), trace=trace
    )
    LAST_EXEC_NS = res.exec_time_ns
    LAST_RESULTS = res

    out = np.empty((T, B, H), np.float32)
    for c in range(NCORES):
        a = np.asarray(res.results[c]["hs"], np.float32).reshape(T, 128, NKH, BL)
        out[:, c * BL:(c + 1) * BL, :] = a.transpose(0, 3, 2, 1).reshape(T, BL, H)
    return out


# revision 26
# speedup vs baseline: 1.6245x; 1.0065x over previous
"""LSTM regression kernel for 8 Trainium2 NeuronCores.

Model (reference): B=2048, IN=2048, H=1024, T=15 steps, x constant across
steps. Data-parallel over batch: each of the 8 cores handles 256 batch rows.

Per-core design (BL=256 batch cols, everything kept transposed [dim, BL]):
 - xg[4H, BL] = W_aug.T @ x_aug computed once (x augmented with a ones row
   that carries b_ih+b_hh), fp16 inputs / f32 PSUM, stored fp16 in SBUF.
 - Step 0 is free of matmuls: h0=c0=0.01 const, so W_hh@h0 = 0.01*rowsum(W_hh)
   is folded into the activation bias (per-partition [128,1] bias per m-tile).
 - Steps 1..14: gates = xg + W_hh @ h_t. The W_hh matmul accumulates in PSUM
   (8 K-chunks of 128, one PSUM bank per gate quarter); the xg add happens on
   VectorE (PSUM+SBUF->SBUF), NOT as identity matmuls - keeps the PE stream
   pure W_hh work (256 N=256 matmuls/step, the fp16 PE roofline).
 - Step boundary: the first two h-chunks' matmuls are staggered (hc0 kc0-6,
   hc1 kc0-6, then both kc7 groups) so the PE has ~6us of runway before it
   reads the previous step's last h-chunks / the last W_hh DMA chunk (t=1).
 - Gate quarters are ordered [i, f, o, g] per h-chunk so ScalarE can run one
   sigmoid over 768 cols + one tanh over 256 cols; the last h-chunk uses
   per-piece activations in (i,g,f,o) order to close its h16 chain sooner.
 - Cell update on VectorE; i,g,o,tanh(c) in fp16 (2x DVE mode), c stays f32.
 - h stored fp16 (feeds next step's matmul and the output DMA; host upcasts).
 - DMA: W_ih streamed as 8 half-MB-scale tile DMAs (first one split for fast
   start), W_hh as 8 chunk DMAs interleaved into the W_ih stream tail; the
   pre-step-1 phase is DMA-bound at ~360GB/s and fully packed.
"""

import os
import numpy as np

try:
    import concourse.bass as bass
except ImportError:  # pragma: no cover
    import sys
    sys.path.insert(0, "/opt/trn_rl_repo")
    import concourse.bass as bass
from concourse import bacc
import concourse.mybir as mybir
import concourse.tile as tile
from concourse.bass_utils import run_bass_kernel_spmd

F32 = mybir.dt.float32
F16 = mybir.dt.float16
AF = mybir.ActivationFunctionType

T = 15
B, IN, H = 2048, 2048, 1024
NCORES = 8
BL = B // NCORES            # 256 batch rows per core
G4 = 4 * H                  # 4096 gate rows
NM = G4 // 128              # 32 gate m-tiles
NKH = H // 128              # 8 hidden K-chunks
NKX = IN // 128             # 16 input K-chunks (bias folded into act/DVE
                            # per-partition bias, no ones-row augmentation)
NHC = NKH                   # 8 h-output chunks
INIT = 0.01
QGATE = (0, 1, 3, 2)        # quarter -> gate index (i, f, o, g)

LAST_EXEC_NS = None
LAST_RESULTS = None

_cached_nc = None


def _build():
    nc = bacc.Bacc(None, target_bir_lowering=False)
    wih = nc.dram_tensor("wih", [NHC, NKX, 128, 4 * 128], F16, kind="ExternalInput")
    whh = nc.dram_tensor("whh", [NKH, 128, G4], F16, kind="ExternalInput")
    xp = nc.dram_tensor("xp", [NKX, 128, BL], F16, kind="ExternalInput")
    bias0 = nc.dram_tensor("bias0", [128, NM], F32, kind="ExternalInput")
    biasg = nc.dram_tensor("biasg", [128, NM], F32, kind="ExternalInput")
    hs = nc.dram_tensor("hs", [T, 128, NKH * BL], F16, kind="ExternalOutput")

    with tile.TileContext(nc) as tc:
        with (
            tc.tile_pool(name="const", bufs=1) as constp,
            tc.tile_pool(name="wihp", bufs=3) as wpool,
            tc.tile_pool(name="state", bufs=2) as statep,
            tc.tile_pool(name="gates", bufs=3) as gp,
            tc.tile_pool(name="psum", bufs=8, space="PSUM") as psump,
        ):
            whh_sb = constp.tile([128, NKH * G4], F16, tag="whh")
            xg_sb = constp.tile([128, NM * BL], F16, tag="xg")
            x_sb = constp.tile([128, NKX * BL], F16, tag="xsb")
            bias_sb = constp.tile([128, NM], F32, tag="bias")
            biasg_sb = constp.tile([128, NM], F32, tag="biasg")

            # x: first K-chunk alone so the first matmul can start ASAP
            nc.sync.dma_start(x_sb[:, 0:BL], xp[0])
            nc.sync.dma_start(
                x_sb[:, BL:].rearrange("p (kc b) -> p kc b", kc=NKX - 1),
                xp[1:, :, :].rearrange("kc p b -> p kc b"),
            )
            nc.sync.dma_start(bias_sb[:, :], bias0[:, :])
            nc.sync.dma_start(biasg_sb[:, :], biasg[:, :])

            def cell_update(hc, ifo, g16, c_prev, c_new, h16):
                sl = slice(hc * BL, (hc + 1) * BL)
                t0 = gp.tile([128, BL], F16, tag="t0")
                t1 = gp.tile([128, BL], F32, tag="t1")
                th = gp.tile([128, BL], F16, tag="th")
                nc.vector.tensor_mul(t0[:, :], ifo[:, 0:BL], g16[:, :])
                nc.vector.tensor_mul(t1[:, :], ifo[:, BL:2 * BL], c_prev[:, sl])
                nc.vector.tensor_add(c_new[:, sl], t0[:, :], t1[:, :])
                nc.scalar.activation(th[:, :], c_new[:, sl], AF.Tanh)
                nc.vector.tensor_mul(h16[:, sl], ifo[:, 2 * BL:3 * BL], th[:, :])

            # ---- xg phase + step 0 (no matmuls for the recurrent part) ----
            c_prev = statep.tile([128, NKH * BL], F32, tag="c")
            nc.vector.memset(c_prev[:, :], INIT)
            h16 = statep.tile([128, NKH * BL], F16, tag="h16")
            c_new = statep.tile([128, NKH * BL], F32, tag="c")

            for hc in range(NHC):
                wt = wpool.tile([128, NKX * 512], F16, tag="wt")
                if hc == 0:
                    # split the first tile's DMA so kc=0 lands quickly
                    for a, b in ((0, 2), (2, 6), (6, 11), (11, NKX)):
                        nc.sync.dma_start(
                            wt[:, a * 512:b * 512].rearrange(
                                "p (kc c) -> p kc c", kc=b - a
                            ),
                            wih[hc, a:b].rearrange("kc p c -> p kc c"),
                        )
                else:
                    nc.sync.dma_start(
                        wt[:, :].rearrange("p (kc c) -> p kc c", kc=NKX),
                        wih[hc].rearrange("kc p c -> p kc c"),
                    )
                ifo = gp.tile([128, 3 * BL], F16, tag="ifo")
                g16 = gp.tile([128, BL], F16, tag="g16")
                for q in range(4):
                    m_abs = QGATE[q] * NKH + hc
                    ps = psump.tile([128, BL], F32, tag="ps")
                    for kc in range(NKX):
                        nc.tensor.matmul(
                            ps[:, :],
                            wt[:, (kc * 4 + q) * 128:(kc * 4 + q + 1) * 128],
                            x_sb[:, kc * BL:(kc + 1) * BL],
                            start=(kc == 0),
                            stop=(kc == NKX - 1),
                        )
                    fn = AF.Tanh if q == 3 else AF.Sigmoid
                    dst = g16[:, :] if q == 3 else ifo[:, q * BL:(q + 1) * BL]
                    nc.scalar.activation(
                        dst, ps[:, :], fn, bias=bias_sb[:, m_abs:m_abs + 1]
                    )
                    nc.vector.tensor_scalar_add(
                        xg_sb[:, (hc * 4 + q) * BL:(hc * 4 + q + 1) * BL],
                        ps[:, :],
                        biasg_sb[:, m_abs:m_abs + 1],
                    )
                cell_update(hc, ifo, g16, c_prev, c_new, h16)
                if hc >= 4:
                    # interleave W_hh chunk loads into the tail of the W_ih
                    # stream (W_hh is only needed from step 1)
                    kc = hc - 4
                    nc.sync.dma_start(whh_sb[:, kc * G4:(kc + 1) * G4], whh[kc])
            for kc in range(4, NKH):
                nc.sync.dma_start(whh_sb[:, kc * G4:(kc + 1) * G4], whh[kc])
            nc.sync.dma_start(hs[0], h16[:, :])
            h_prev, c_prev = h16, c_new

            # ---- recurrent steps 1..14 ----
            def mm_quarter(ps, hc, q, kc, start, stop):
                m_abs = QGATE[q] * NKH + hc
                nc.tensor.matmul(
                    ps[:, :],
                    whh_sb[:, kc * G4 + m_abs * 128: kc * G4 + (m_abs + 1) * 128],
                    h_prev[:, kc * BL:(kc + 1) * BL],
                    start=start,
                    stop=stop,
                )

            def gates_and_cell(hc, pstiles, c_prev, c_new, h16, last):
                g32 = gp.tile([128, 4 * BL], F32, tag="g32")
                ifo = gp.tile([128, 3 * BL], F16, tag="ifo")
                g16 = gp.tile([128, BL], F16, tag="g16")

                def add_q(q):
                    nc.vector.tensor_add(
                        g32[:, q * BL:(q + 1) * BL],
                        pstiles[q][:, :],
                        xg_sb[:, (hc * 4 + q) * BL:(hc * 4 + q + 1) * BL],
                    )

                if last:
                    # per-piece activations in (i, g, f, o) order so the h16
                    # chain closes sooner
                    for q in (0, 3, 1, 2):
                        add_q(q)
                        fn = AF.Tanh if q == 3 else AF.Sigmoid
                        dst = (g16[:, :] if q == 3
                               else ifo[:, q * BL:(q + 1) * BL])
                        nc.scalar.activation(dst, g32[:, q * BL:(q + 1) * BL], fn)
                else:
                    for q in range(4):
                        add_q(q)
                    nc.scalar.activation(ifo[:, :], g32[:, 0:3 * BL], AF.Sigmoid)
                    nc.scalar.activation(g16[:, :], g32[:, 3 * BL:4 * BL], AF.Tanh)
                cell_update(hc, ifo, g16, c_prev, c_new, h16)

            for t in range(1, T):
                h16 = statep.tile([128, NKH * BL], F16, tag="h16")
                c_new = statep.tile([128, NKH * BL], F32, tag="c")
                # head (hc0+hc1): stagger so reads of the previous step's
                # late chunks are pushed out - hc0 kc0-5, hc0 kc6, hc1 kc0-6,
                # then the kc7 reads only after ~6us of runway (also covers
                # the last W_hh DMA chunk still in flight at t=1)
                pst0 = {}
                pst1 = {}
                nq1 = 3 if t == 1 else 4
                for q in range(4):
                    ps = psump.tile([128, BL], F32, tag="ps", name=f"ps0_{q}")
                    pst0[q] = ps
                    for kc in range(NKH - 2):
                        mm_quarter(ps, 0, q, kc, start=(kc == 0), stop=False)
                for q in range(4):
                    mm_quarter(pst0[q], 0, q, NKH - 2, start=False, stop=False)
                for q in range(nq1):
                    ps = psump.tile([128, BL], F32, tag="ps", name=f"ps1_{q}")
                    pst1[q] = ps
                    for kc in range(NKH - 1):
                        mm_quarter(ps, 1, q, kc, start=(kc == 0), stop=False)
                if t == 1:
                    # the last W_hh chunk's DMA is still in flight here; keep
                    # the PE p-state warm with throwaway matmuls into a scratch
                    # bank (results never read) while the kc=7 closers wait
                    warm = psump.tile([128, BL], F32, tag="ps", name="ps_warm")
                    for _ in range(14):
                        nc.tensor.matmul(
                            warm[:, :], whh_sb[:, 0:128], h_prev[:, 0:BL],
                            start=True, stop=True,
                        )
                for q in range(4):
                    mm_quarter(pst0[q], 0, q, NKH - 1, start=False, stop=True)
                for q in range(nq1):
                    mm_quarter(pst1[q], 1, q, NKH - 1, start=False, stop=True)
                if t == 1:
                    # hc1-q3 ran zero matmuls so far; do its full group now
                    ps = psump.tile([128, BL], F32, tag="ps", name="ps1_3")
                    pst1[3] = ps
                    for kc in range(NKH):
                        mm_quarter(ps, 1, 3, kc,
                                   start=(kc == 0), stop=(kc == NKH - 1))
                gates_and_cell(0, pst0, c_prev, c_new, h16, last=False)
                gates_and_cell(1, pst1, c_prev, c_new, h16, last=False)
                for hc in range(2, NHC):
                    last = hc == NHC - 1
                    pstiles = {}
                    qseq = (0, 3, 1, 2) if last else (0, 1, 2, 3)
                    for q in qseq:
                        ps = psump.tile([128, BL], F32, tag="ps")
                        pstiles[q] = ps
                        for kc in range(NKH):
                            mm_quarter(ps, hc, q, kc,
                                       start=(kc == 0), stop=(kc == NKH - 1))
                    gates_and_cell(hc, pstiles, c_prev, c_new, h16, last)
                    if t == T - 1:
                        # last step: stream out chunks as they complete so the
                        # kernel tail isn't one big dependent DMA
                        if hc == 2:
                            nc.sync.dma_start(hs[t, :, 0:2 * BL], h16[:, 0:2 * BL])
                        if hc >= 2:
                            nc.sync.dma_start(
                                hs[t, :, hc * BL:(hc + 1) * BL],
                                h16[:, hc * BL:(hc + 1) * BL],
                            )
                if t < T - 1:
                    nc.sync.dma_start(hs[t], h16[:, :])
                h_prev, c_prev = h16, c_new

    nc.compile()
    return nc


def timeline_ns():
    from concourse.timeline_sim import TimelineSim
    nc = _get_nc()
    ts = TimelineSim(nc)
    ts.simulate()
    return ts.time


def _get_nc():
    global _cached_nc
    if _cached_nc is None:
        _cached_nc = _build()
    return _cached_nc


def kernel(x, W_ih, W_hh, b_ih, b_hh):
    global LAST_EXEC_NS, LAST_RESULTS
    nc = _get_nc()
    x = np.asarray(x, np.float32)
    W_ih = np.asarray(W_ih, np.float32)
    W_hh = np.asarray(W_hh, np.float32)
    b_ih = np.asarray(b_ih, np.float32)
    b_hh = np.asarray(b_hh, np.float32)

    # [kc][p][gi][hc][c] -> quarter order (i,f,o,g) -> [hc][kc][p][q*128+c]
    wih_pack = np.ascontiguousarray(
        W_ih.T.reshape(NKX, 128, 4, NKH, 128)[:, :, QGATE, :, :]
        .transpose(3, 0, 1, 2, 4)
        .reshape(NHC, NKX, 128, 512)
    ).astype(np.float16)
    whh_pack = np.ascontiguousarray(W_hh.T.reshape(NKH, 128, G4)).astype(np.float16)
    bg = (b_ih + b_hh).astype(np.float32)
    biasg_pack = np.ascontiguousarray(bg.reshape(NM, 128).T).astype(np.float32)
    bias_pack = np.ascontiguousarray(
        (INIT * W_hh.sum(1) + bg).reshape(NM, 128).T
    ).astype(np.float32)

    in_maps = []
    for c in range(NCORES):
        xa = np.ascontiguousarray(x[c * BL:(c + 1) * BL].T)
        in_maps.append({
            "wih": wih_pack,
            "whh": whh_pack,
            "xp": xa.reshape(NKX, 128, BL).astype(np.float16),
            "bias0": bias_pack,
            "biasg": biasg_pack,
        })

    trace = os.environ.get("LSTM_TRACE") == "1"
    res = run_bass_kernel_spmd(
        nc, in_maps, core_ids=list(range(NCORES)), trace=trace
    )
    LAST_EXEC_NS = res.exec_time_ns
    LAST_RESULTS = res

    out = np.empty((T, B, H), np.float32)
    for c in range(NCORES):
        a = np.asarray(res.results[c]["hs"], np.float32).reshape(T, 128, NKH, BL)
        out[:, c * BL:(c + 1) * BL, :] = a.transpose(0, 3, 2, 1).reshape(T, BL, H)
    return out


# revision 29
# speedup vs baseline: 1.6255x; 1.0006x over previous
"""LSTM regression kernel for 8 Trainium2 NeuronCores.

Model (reference): B=2048, IN=2048, H=1024, T=15 steps, x constant across
steps. Data-parallel over batch: each of the 8 cores handles 256 batch rows.

Per-core design (BL=256 batch cols, everything kept transposed [dim, BL]):
 - xg[4H, BL] = W_aug.T @ x_aug computed once (x augmented with a ones row
   that carries b_ih+b_hh), fp16 inputs / f32 PSUM, stored fp16 in SBUF.
 - Step 0 is free of matmuls: h0=c0=0.01 const, so W_hh@h0 = 0.01*rowsum(W_hh)
   is folded into the activation bias (per-partition [128,1] bias per m-tile).
 - Steps 1..14: gates = xg + W_hh @ h_t. The W_hh matmul accumulates in PSUM
   (8 K-chunks of 128, one PSUM bank per gate quarter); the xg add happens on
   VectorE (PSUM+SBUF->SBUF), NOT as identity matmuls - keeps the PE stream
   pure W_hh work (256 N=256 matmuls/step, the fp16 PE roofline).
 - Step boundary: the first two h-chunks' matmuls are staggered (hc0 kc0-6,
   hc1 kc0-6, then both kc7 groups) so the PE has ~6us of runway before it
   reads the previous step's last h-chunks / the last W_hh DMA chunk (t=1).
 - Gate quarters are ordered [i, f, o, g] per h-chunk so ScalarE can run one
   sigmoid over 768 cols + one tanh over 256 cols; the last h-chunk uses
   per-piece activations in (i,g,f,o) order to close its h16 chain sooner.
 - Cell update on VectorE; i,g,o,tanh(c) in fp16 (2x DVE mode), c stays f32.
 - h stored fp16 (feeds next step's matmul and the output DMA; host upcasts).
 - DMA: W_ih streamed as 8 half-MB-scale tile DMAs (first one split for fast
   start), W_hh as 8 chunk DMAs interleaved into the W_ih stream tail; the
   pre-step-1 phase is DMA-bound at ~360GB/s and fully packed.
"""

import os
import numpy as np

try:
    import concourse.bass as bass
except ImportError:  # pragma: no cover
    import sys
    sys.path.insert(0, "/opt/trn_rl_repo")
    import concourse.bass as bass
from concourse import bacc
import concourse.mybir as mybir
import concourse.tile as tile
from concourse.bass_utils import run_bass_kernel_spmd
from concourse.masks import make_identity

F32 = mybir.dt.float32
F16 = mybir.dt.float16
AF = mybir.ActivationFunctionType

T = 15
B, IN, H = 2048, 2048, 1024
NCORES = 8
BL = B // NCORES            # 256 batch rows per core
G4 = 4 * H                  # 4096 gate rows
NM = G4 // 128              # 32 gate m-tiles
NKH = H // 128              # 8 hidden K-chunks
NKX = IN // 128             # 16 input K-chunks (bias folded into act/DVE
                            # per-partition bias, no ones-row augmentation)
NHC = NKH                   # 8 h-output chunks
INIT = 0.01
QGATE = (0, 1, 3, 2)        # quarter -> gate index (i, f, o, g)

LAST_EXEC_NS = None
LAST_RESULTS = None

_cached_nc = None


def _build():
    nc = bacc.Bacc(None, target_bir_lowering=False)
    wih = nc.dram_tensor("wih", [NHC, NKX, 128, 4 * 128], F16, kind="ExternalInput")
    whh = nc.dram_tensor("whh", [NKH, 128, G4], F16, kind="ExternalInput")
    xp = nc.dram_tensor("xp", [NKX, 128, BL], F16, kind="ExternalInput")
    bias0 = nc.dram_tensor("bias0", [128, NM], F32, kind="ExternalInput")
    biasg = nc.dram_tensor("biasg", [128, NM], F32, kind="ExternalInput")
    hs = nc.dram_tensor("hs", [T, 128, NKH * BL], F16, kind="ExternalOutput")

    with tile.TileContext(nc) as tc:
        with (
            tc.tile_pool(name="const", bufs=1) as constp,
            tc.tile_pool(name="wihp", bufs=3) as wpool,
            tc.tile_pool(name="state", bufs=2) as statep,
            tc.tile_pool(name="gates", bufs=3) as gp,
            tc.tile_pool(name="psum", bufs=8, space="PSUM") as psump,
        ):
            whh_sb = constp.tile([128, NKH * G4], F16, tag="whh")
            xg_sb = constp.tile([128, NM * BL], F16, tag="xg")
            x_sb = constp.tile([128, NKX * BL], F16, tag="xsb")
            bias_sb = constp.tile([128, NM], F32, tag="bias")
            biasg_sb = constp.tile([128, NM], F32, tag="biasg")
            ident = constp.tile([128, 128], F16, tag="ident")
            make_identity(nc, ident[:, :])

            # x: first K-chunk alone so the first matmul can start ASAP
            nc.sync.dma_start(x_sb[:, 0:BL], xp[0])
            nc.sync.dma_start(
                x_sb[:, BL:].rearrange("p (kc b) -> p kc b", kc=NKX - 1),
                xp[1:, :, :].rearrange("kc p b -> p kc b"),
            )
            nc.sync.dma_start(bias_sb[:, :], bias0[:, :])
            nc.sync.dma_start(biasg_sb[:, :], biasg[:, :])

            def cell_update(hc, ifo, g16, c_prev, c_new, h16):
                sl = slice(hc * BL, (hc + 1) * BL)
                t0 = gp.tile([128, BL], F16, tag="t0")
                t1 = gp.tile([128, BL], F32, tag="t1")
                th = gp.tile([128, BL], F16, tag="th")
                nc.vector.tensor_mul(t0[:, :], ifo[:, 0:BL], g16[:, :])
                nc.vector.tensor_mul(t1[:, :], ifo[:, BL:2 * BL], c_prev[:, sl])
                nc.vector.tensor_add(c_new[:, sl], t0[:, :], t1[:, :])
                nc.scalar.activation(th[:, :], c_new[:, sl], AF.Tanh)
                nc.vector.tensor_mul(h16[:, sl], ifo[:, 2 * BL:3 * BL], th[:, :])

            # ---- xg phase + step 0 (no matmuls for the recurrent part) ----
            c_prev = statep.tile([128, NKH * BL], F32, tag="c")
            nc.vector.memset(c_prev[:, :], INIT)
            h16 = statep.tile([128, NKH * BL], F16, tag="h16")
            c_new = statep.tile([128, NKH * BL], F32, tag="c")

            for hc in range(NHC):
                wt = wpool.tile([128, NKX * 512], F16, tag="wt")
                if hc == 0:
                    # split the first tile's DMA so kc=0 lands quickly
                    for a, b in ((0, 2), (2, 6), (6, 11), (11, NKX)):
                        nc.sync.dma_start(
                            wt[:, a * 512:b * 512].rearrange(
                                "p (kc c) -> p kc c", kc=b - a
                            ),
                            wih[hc, a:b].rearrange("kc p c -> p kc c"),
                        )
                else:
                    nc.sync.dma_start(
                        wt[:, :].rearrange("p (kc c) -> p kc c", kc=NKX),
                        wih[hc].rearrange("kc p c -> p kc c"),
                    )
                ifo = gp.tile([128, 3 * BL], F16, tag="ifo")
                g16 = gp.tile([128, BL], F16, tag="g16")
                for q in range(4):
                    m_abs = QGATE[q] * NKH + hc
                    ps = psump.tile([128, BL], F32, tag="ps")
                    for kc in range(NKX):
                        nc.tensor.matmul(
                            ps[:, :],
                            wt[:, (kc * 4 + q) * 128:(kc * 4 + q + 1) * 128],
                            x_sb[:, kc * BL:(kc + 1) * BL],
                            start=(kc == 0),
                            stop=(kc == NKX - 1),
                        )
                    fn = AF.Tanh if q == 3 else AF.Sigmoid
                    dst = g16[:, :] if q == 3 else ifo[:, q * BL:(q + 1) * BL]
                    nc.scalar.activation(
                        dst, ps[:, :], fn, bias=bias_sb[:, m_abs:m_abs + 1]
                    )
                    nc.vector.tensor_scalar_add(
                        xg_sb[:, (hc * 4 + q) * BL:(hc * 4 + q + 1) * BL],
                        ps[:, :],
                        biasg_sb[:, m_abs:m_abs + 1],
                    )
                cell_update(hc, ifo, g16, c_prev, c_new, h16)
                if hc >= 4:
                    # interleave W_hh chunk loads into the tail of the W_ih
                    # stream (W_hh is only needed from step 1)
                    kc = hc - 4
                    nc.sync.dma_start(whh_sb[:, kc * G4:(kc + 1) * G4], whh[kc])
            for kc in range(4, NKH):
                nc.sync.dma_start(whh_sb[:, kc * G4:(kc + 1) * G4], whh[kc])
            nc.sync.dma_start(hs[0], h16[:, :])
            h_prev, c_prev = h16, c_new

            # ---- recurrent steps 1..14 ----
            def mm_quarter(ps, hc, q, kc, start, stop):
                m_abs = QGATE[q] * NKH + hc
                nc.tensor.matmul(
                    ps[:, :],
                    whh_sb[:, kc * G4 + m_abs * 128: kc * G4 + (m_abs + 1) * 128],
                    h_prev[:, kc * BL:(kc + 1) * BL],
                    start=start,
                    stop=stop,
                )

            def gates_and_cell(hc, pstiles, c_prev, c_new, h16, last,
                               fuse_xg_mm=False):
                ifo = gp.tile([128, 3 * BL], F16, tag="ifo")
                g16 = gp.tile([128, BL], F16, tag="g16")
                g32 = None if fuse_xg_mm else gp.tile(
                    [128, 4 * BL], F32, tag="g32", name="g32"
                )

                def add_q(q):
                    nc.vector.tensor_add(
                        g32[:, q * BL:(q + 1) * BL],
                        pstiles[q][:, :],
                        xg_sb[:, (hc * 4 + q) * BL:(hc * 4 + q + 1) * BL],
                    )

                if fuse_xg_mm:
                    # kernel tail only (PE is otherwise idle): add xg via an
                    # identity matmul into the still-open PSUM group, so the
                    # activations read PSUM directly and no DVE add sits on
                    # the final h16 chain
                    for q in (0, 3, 1, 2):
                        nc.tensor.matmul(
                            pstiles[q][:, :],
                            ident[:, :],
                            xg_sb[:, (hc * 4 + q) * BL:(hc * 4 + q + 1) * BL],
                            start=False,
                            stop=True,
                        )
                        fn = AF.Tanh if q == 3 else AF.Sigmoid
                        dst = (g16[:, :] if q == 3
                               else ifo[:, q * BL:(q + 1) * BL])
                        nc.scalar.activation(dst, pstiles[q][:, :], fn)
                elif last:
                    # per-piece activations in (i, g, f, o) order so the h16
                    # chain closes sooner
                    for q in (0, 3, 1, 2):
                        add_q(q)
                        fn = AF.Tanh if q == 3 else AF.Sigmoid
                        dst = (g16[:, :] if q == 3
                               else ifo[:, q * BL:(q + 1) * BL])
                        nc.scalar.activation(dst, g32[:, q * BL:(q + 1) * BL], fn)
                else:
                    for q in range(4):
                        add_q(q)
                    nc.scalar.activation(ifo[:, :], g32[:, 0:3 * BL], AF.Sigmoid)
                    nc.scalar.activation(g16[:, :], g32[:, 3 * BL:4 * BL], AF.Tanh)
                cell_update(hc, ifo, g16, c_prev, c_new, h16)

            for t in range(1, T):
                h16 = statep.tile([128, NKH * BL], F16, tag="h16")
                c_new = statep.tile([128, NKH * BL], F32, tag="c")
                # head (hc0+hc1): stagger so reads of the previous step's
                # late chunks are pushed out - hc0 kc0-5, hc0 kc6, hc1 kc0-6,
                # then the kc7 reads only after ~6us of runway (also covers
                # the last W_hh DMA chunk still in flight at t=1)
                pst0 = {}
                pst1 = {}
                nq1 = 3 if t == 1 else 4
                for q in range(4):
                    ps = psump.tile([128, BL], F32, tag="ps", name=f"ps0_{q}")
                    pst0[q] = ps
                    for kc in range(NKH - 2):
                        mm_quarter(ps, 0, q, kc, start=(kc == 0), stop=False)
                for q in range(4):
                    mm_quarter(pst0[q], 0, q, NKH - 2, start=False, stop=False)
                for q in range(nq1):
                    ps = psump.tile([128, BL], F32, tag="ps", name=f"ps1_{q}")
                    pst1[q] = ps
                    for kc in range(NKH - 1):
                        mm_quarter(ps, 1, q, kc, start=(kc == 0), stop=False)
                if t == 1:
                    # the last W_hh chunk's DMA is still in flight here; keep
                    # the PE p-state warm with throwaway matmuls into a scratch
                    # bank (results never read) while the kc=7 closers wait
                    warm = psump.tile([128, BL], F32, tag="ps", name="ps_warm")
                    for _ in range(14):
                        nc.tensor.matmul(
                            warm[:, :], whh_sb[:, 0:128], h_prev[:, 0:BL],
                            start=True, stop=True,
                        )
                for q in range(4):
                    mm_quarter(pst0[q], 0, q, NKH - 1, start=False, stop=True)
                for q in range(nq1):
                    mm_quarter(pst1[q], 1, q, NKH - 1, start=False, stop=True)
                if t == 1:
                    # hc1-q3 ran zero matmuls so far; do its full group now
                    ps = psump.tile([128, BL], F32, tag="ps", name="ps1_3")
                    pst1[3] = ps
                    for kc in range(NKH):
                        mm_quarter(ps, 1, 3, kc,
                                   start=(kc == 0), stop=(kc == NKH - 1))
                gates_and_cell(0, pst0, c_prev, c_new, h16, last=False)
                gates_and_cell(1, pst1, c_prev, c_new, h16, last=False)
                for hc in range(2, NHC):
                    last = hc == NHC - 1
                    fuse = last and t == T - 1
                    pstiles = {}
                    qseq = (0, 3, 1, 2) if last else (0, 1, 2, 3)
                    for q in qseq:
                        ps = psump.tile([128, BL], F32, tag="ps")
                        pstiles[q] = ps
                        for kc in range(NKH):
                            mm_quarter(ps, hc, q, kc,
                                       start=(kc == 0),
                                       stop=(kc == NKH - 1 and not fuse))
                    gates_and_cell(hc, pstiles, c_prev, c_new, h16, last,
                                   fuse_xg_mm=fuse)
                    if t == T - 1:
                        # last step: stream out chunks as they complete so the
                        # kernel tail isn't one big dependent DMA
                        if hc == 2:
                            nc.sync.dma_start(hs[t, :, 0:2 * BL], h16[:, 0:2 * BL])
                        if hc >= 2:
                            nc.sync.dma_start(
                                hs[t, :, hc * BL:(hc + 1) * BL],
                                h16[:, hc * BL:(hc + 1) * BL],
                            )
                if t < T - 1:
                    nc.sync.dma_start(hs[t], h16[:, :])
                h_prev, c_prev = h16, c_new

    nc.compile()
    return nc


def timeline_ns():
    from concourse.timeline_sim import TimelineSim
    nc = _get_nc()
    ts = TimelineSim(nc)
    ts.simulate()
    return ts.time


def _get_nc():
    global _cached_nc
    if _cached_nc is None:
        _cached_nc = _build()
    return _cached_nc


def kernel(x, W_ih, W_hh, b_ih, b_hh):
    global LAST_EXEC_NS, LAST_RESULTS
    nc = _get_nc()
    x = np.asarray(x, np.float32)
    W_ih = np.asarray(W_ih, np.float32)
    W_hh = np.asarray(W_hh, np.float32)
    b_ih = np.asarray(b_ih, np.float32)
    b_hh = np.asarray(b_hh, np.float32)

    # [kc][p][gi][hc][c] -> quarter order (i,f,o,g) -> [hc][kc][p][q*128+c]
    wih_pack = np.ascontiguousarray(
        W_ih.T.reshape(NKX, 128, 4, NKH, 128)[:, :, QGATE, :, :]
        .transpose(3, 0, 1, 2, 4)
        .reshape(NHC, NKX, 128, 512)
    ).astype(np.float16)
    whh_pack = np.ascontiguousarray(W_hh.T.reshape(NKH, 128, G4)).astype(np.float16)
    bg = (b_ih + b_hh).astype(np.float32)
    biasg_pack = np.ascontiguousarray(bg.reshape(NM, 128).T).astype(np.float32)
    bias_pack = np.ascontiguousarray(
        (INIT * W_hh.sum(1) + bg).reshape(NM, 128).T
    ).astype(np.float32)

    in_maps = []
    for c in range(NCORES):
        xa = np.ascontiguousarray(x[c * BL:(c + 1) * BL].T)
        in_maps.append({
            "wih": wih_pack,
            "whh": whh_pack,
            "xp": xa.reshape(NKX, 128, BL).astype(np.float16),
            "bias0": bias_pack,
            "biasg": biasg_pack,
        })

    trace = os.environ.get("LSTM_TRACE") == "1"
    res = run_bass_kernel_spmd(
        nc, in_maps, core_ids=list(range(NCORES)), trace=trace
    )
    LAST_EXEC_NS = res.exec_time_ns
    LAST_RESULTS = res

    out = np.empty((T, B, H), np.float32)
    for c in range(NCORES):
        a = np.asarray(res.results[c]["hs"], np.float32).reshape(T, 128, NKH, BL)
        out[:, c * BL:(c + 1) * BL, :] = a.transpose(0, 3, 2, 1).reshape(T, BL, H)
    return out


# revision 30
# speedup vs baseline: 1.6268x; 1.0008x over previous
"""LSTM regression kernel for 8 Trainium2 NeuronCores.

Model (reference): B=2048, IN=2048, H=1024, T=15 steps, x constant across
steps. Data-parallel over batch: each of the 8 cores handles 256 batch rows.

Per-core design (BL=256 batch cols, everything kept transposed [dim, BL]):
 - xg[4H, BL] = W_aug.T @ x_aug computed once (x augmented with a ones row
   that carries b_ih+b_hh), fp16 inputs / f32 PSUM, stored fp16 in SBUF.
 - Step 0 is free of matmuls: h0=c0=0.01 const, so W_hh@h0 = 0.01*rowsum(W_hh)
   is folded into the activation bias (per-partition [128,1] bias per m-tile).
 - Steps 1..14: gates = xg + W_hh @ h_t. The W_hh matmul accumulates in PSUM
   (8 K-chunks of 128, one PSUM bank per gate quarter); the xg add happens on
   VectorE (PSUM+SBUF->SBUF), NOT as identity matmuls - keeps the PE stream
   pure W_hh work (256 N=256 matmuls/step, the fp16 PE roofline).
 - Step boundary: the first two h-chunks' matmuls are staggered (hc0 kc0-6,
   hc1 kc0-6, then both kc7 groups) so the PE has ~6us of runway before it
   reads the previous step's last h-chunks / the last W_hh DMA chunk (t=1).
 - Gate quarters are ordered [i, f, o, g] per h-chunk so ScalarE can run one
   sigmoid over 768 cols + one tanh over 256 cols; the last h-chunk uses
   per-piece activations in (i,g,f,o) order to close its h16 chain sooner.
 - Cell update on VectorE; i,g,o,tanh(c) in fp16 (2x DVE mode), c stays f32.
 - h stored fp16 (feeds next step's matmul and the output DMA; host upcasts).
 - DMA: W_ih streamed as 8 half-MB-scale tile DMAs (first one split for fast
   start), W_hh as 8 chunk DMAs interleaved into the W_ih stream tail; the
   pre-step-1 phase is DMA-bound at ~360GB/s and fully packed.
"""

import os
import numpy as np

try:
    import concourse.bass as bass
except ImportError:  # pragma: no cover
    import sys
    sys.path.insert(0, "/opt/trn_rl_repo")
    import concourse.bass as bass
from concourse import bacc
import concourse.mybir as mybir
import concourse.tile as tile
from concourse.bass_utils import run_bass_kernel_spmd
from concourse.masks import make_identity

F32 = mybir.dt.float32
F16 = mybir.dt.float16
AF = mybir.ActivationFunctionType

T = 15
B, IN, H = 2048, 2048, 1024
NCORES = 8
BL = B // NCORES            # 256 batch rows per core
G4 = 4 * H                  # 4096 gate rows
NM = G4 // 128              # 32 gate m-tiles
NKH = H // 128              # 8 hidden K-chunks
NKX = IN // 128             # 16 input K-chunks (bias folded into act/DVE
                            # per-partition bias, no ones-row augmentation)
NHC = NKH                   # 8 h-output chunks
INIT = 0.01
QGATE = (0, 1, 3, 2)        # quarter -> gate index (i, f, o, g)

LAST_EXEC_NS = None
LAST_RESULTS = None

_cached_nc = None


def _build():
    nc = bacc.Bacc(None, target_bir_lowering=False)
    wih = nc.dram_tensor("wih", [NHC, NKX, 128, 4 * 128], F16, kind="ExternalInput")
    whh = nc.dram_tensor("whh", [NKH, 128, G4], F16, kind="ExternalInput")
    xp = nc.dram_tensor("xp", [NKX, 128, BL], F16, kind="ExternalInput")
    bias0 = nc.dram_tensor("bias0", [128, NM], F32, kind="ExternalInput")
    biasg = nc.dram_tensor("biasg", [128, NM], F32, kind="ExternalInput")
    hs = nc.dram_tensor("hs", [T, 128, NKH * BL], F16, kind="ExternalOutput")

    with tile.TileContext(nc) as tc:
        with (
            tc.tile_pool(name="const", bufs=1) as constp,
            tc.tile_pool(name="wihp", bufs=3) as wpool,
            tc.tile_pool(name="state", bufs=2) as statep,
            tc.tile_pool(name="gates", bufs=3) as gp,
            tc.tile_pool(name="psum", bufs=8, space="PSUM") as psump,
        ):
            whh_sb = constp.tile([128, NKH * G4], F16, tag="whh")
            xg_sb = constp.tile([128, NM * BL], F16, tag="xg")
            x_sb = constp.tile([128, NKX * BL], F16, tag="xsb")
            bias_sb = constp.tile([128, NM], F32, tag="bias")
            biasg_sb = constp.tile([128, NM], F32, tag="biasg")
            ident = constp.tile([128, 128], F16, tag="ident")
            make_identity(nc, ident[:, :])

            # x: first K-chunk alone so the first matmul can start ASAP
            nc.sync.dma_start(x_sb[:, 0:BL], xp[0])
            nc.sync.dma_start(
                x_sb[:, BL:].rearrange("p (kc b) -> p kc b", kc=NKX - 1),
                xp[1:, :, :].rearrange("kc p b -> p kc b"),
            )
            nc.sync.dma_start(bias_sb[:, :], bias0[:, :])
            nc.sync.dma_start(biasg_sb[:, :], biasg[:, :])

            def cell_update(hc, ifo, g16, c_prev, c_new, h16):
                sl = slice(hc * BL, (hc + 1) * BL)
                t0 = gp.tile([128, BL], F16, tag="t0")
                t1 = gp.tile([128, BL], F32, tag="t1")
                th = gp.tile([128, BL], F16, tag="th")
                nc.vector.tensor_mul(t0[:, :], ifo[:, 0:BL], g16[:, :])
                nc.vector.tensor_mul(t1[:, :], ifo[:, BL:2 * BL], c_prev[:, sl])
                nc.vector.tensor_add(c_new[:, sl], t0[:, :], t1[:, :])
                nc.scalar.activation(th[:, :], c_new[:, sl], AF.Tanh)
                nc.vector.tensor_mul(h16[:, sl], ifo[:, 2 * BL:3 * BL], th[:, :])

            # ---- xg phase + step 0 (no matmuls for the recurrent part) ----
            c_prev = statep.tile([128, NKH * BL], F32, tag="c")
            nc.vector.memset(c_prev[:, :], INIT)
            h16 = statep.tile([128, NKH * BL], F16, tag="h16")
            c_new = statep.tile([128, NKH * BL], F32, tag="c")

            for hc in range(NHC):
                wt = wpool.tile([128, NKX * 512], F16, tag="wt")
                if hc == 0:
                    # split the first tile's DMA so kc=0 lands quickly
                    for a, b in ((0, 2), (2, 6), (6, 11), (11, NKX)):
                        nc.sync.dma_start(
                            wt[:, a * 512:b * 512].rearrange(
                                "p (kc c) -> p kc c", kc=b - a
                            ),
                            wih[hc, a:b].rearrange("kc p c -> p kc c"),
                        )
                else:
                    nc.sync.dma_start(
                        wt[:, :].rearrange("p (kc c) -> p kc c", kc=NKX),
                        wih[hc].rearrange("kc p c -> p kc c"),
                    )
                ifo = gp.tile([128, 3 * BL], F16, tag="ifo")
                g16 = gp.tile([128, BL], F16, tag="g16")
                for q in range(4):
                    m_abs = QGATE[q] * NKH + hc
                    ps = psump.tile([128, BL], F32, tag="ps")
                    for kc in range(NKX):
                        nc.tensor.matmul(
                            ps[:, :],
                            wt[:, (kc * 4 + q) * 128:(kc * 4 + q + 1) * 128],
                            x_sb[:, kc * BL:(kc + 1) * BL],
                            start=(kc == 0),
                            stop=(kc == NKX - 1),
                        )
                    fn = AF.Tanh if q == 3 else AF.Sigmoid
                    dst = g16[:, :] if q == 3 else ifo[:, q * BL:(q + 1) * BL]
                    nc.scalar.activation(
                        dst, ps[:, :], fn, bias=bias_sb[:, m_abs:m_abs + 1]
                    )
                    nc.vector.tensor_scalar_add(
                        xg_sb[:, (hc * 4 + q) * BL:(hc * 4 + q + 1) * BL],
                        ps[:, :],
                        biasg_sb[:, m_abs:m_abs + 1],
                    )
                cell_update(hc, ifo, g16, c_prev, c_new, h16)
                if hc >= 4:
                    # interleave W_hh chunk loads into the tail of the W_ih
                    # stream (W_hh is only needed from step 1)
                    kc = hc - 4
                    nc.sync.dma_start(whh_sb[:, kc * G4:(kc + 1) * G4], whh[kc])
            for kc in range(4, NKH - 1):
                nc.sync.dma_start(whh_sb[:, kc * G4:(kc + 1) * G4], whh[kc])
            kc = NKH - 1
            for r in range(4):
                # last chunk in 4 m-range pieces: the t=1 closers only need
                # their own gate's slice, so they unblock piecewise
                nc.sync.dma_start(
                    whh_sb[:, kc * G4 + r * 1024:kc * G4 + (r + 1) * 1024],
                    whh[kc][:, r * 1024:(r + 1) * 1024],
                )
            nc.sync.dma_start(hs[0], h16[:, :])
            h_prev, c_prev = h16, c_new

            # ---- recurrent steps 1..14 ----
            def mm_quarter(ps, hc, q, kc, start, stop):
                m_abs = QGATE[q] * NKH + hc
                nc.tensor.matmul(
                    ps[:, :],
                    whh_sb[:, kc * G4 + m_abs * 128: kc * G4 + (m_abs + 1) * 128],
                    h_prev[:, kc * BL:(kc + 1) * BL],
                    start=start,
                    stop=stop,
                )

            def gates_and_cell(hc, pstiles, c_prev, c_new, h16, last,
                               fuse_xg_mm=False):
                ifo = gp.tile([128, 3 * BL], F16, tag="ifo")
                g16 = gp.tile([128, BL], F16, tag="g16")
                g32 = None if fuse_xg_mm else gp.tile(
                    [128, 4 * BL], F32, tag="g32", name="g32"
                )

                def add_q(q):
                    nc.vector.tensor_add(
                        g32[:, q * BL:(q + 1) * BL],
                        pstiles[q][:, :],
                        xg_sb[:, (hc * 4 + q) * BL:(hc * 4 + q + 1) * BL],
                    )

                if fuse_xg_mm:
                    # kernel tail only (PE is otherwise idle): add xg via an
                    # identity matmul into the still-open PSUM group, so the
                    # activations read PSUM directly and no DVE add sits on
                    # the final h16 chain
                    for q in (0, 3, 1, 2):
                        nc.tensor.matmul(
                            pstiles[q][:, :],
                            ident[:, :],
                            xg_sb[:, (hc * 4 + q) * BL:(hc * 4 + q + 1) * BL],
                            start=False,
                            stop=True,
                        )
                        fn = AF.Tanh if q == 3 else AF.Sigmoid
                        dst = (g16[:, :] if q == 3
                               else ifo[:, q * BL:(q + 1) * BL])
                        nc.scalar.activation(dst, pstiles[q][:, :], fn)
                elif last:
                    # per-piece activations in (i, g, f, o) order so the h16
                    # chain closes sooner
                    for q in (0, 3, 1, 2):
                        add_q(q)
                        fn = AF.Tanh if q == 3 else AF.Sigmoid
                        dst = (g16[:, :] if q == 3
                               else ifo[:, q * BL:(q + 1) * BL])
                        nc.scalar.activation(dst, g32[:, q * BL:(q + 1) * BL], fn)
                else:
                    for q in range(4):
                        add_q(q)
                    nc.scalar.activation(ifo[:, :], g32[:, 0:3 * BL], AF.Sigmoid)
                    nc.scalar.activation(g16[:, :], g32[:, 3 * BL:4 * BL], AF.Tanh)
                cell_update(hc, ifo, g16, c_prev, c_new, h16)

            for t in range(1, T):
                h16 = statep.tile([128, NKH * BL], F16, tag="h16")
                c_new = statep.tile([128, NKH * BL], F32, tag="c")
                # head (hc0+hc1): stagger so reads of the previous step's
                # late chunks are pushed out - hc0 kc0-5, hc0 kc6, hc1 kc0-6,
                # then the kc7 reads only after ~6us of runway (also covers
                # the last W_hh DMA chunk still in flight at t=1)
                pst0 = {}
                pst1 = {}
                nq1 = 3 if t == 1 else 4
                for q in range(4):
                    ps = psump.tile([128, BL], F32, tag="ps", name=f"ps0_{q}")
                    pst0[q] = ps
                    for kc in range(NKH - 2):
                        mm_quarter(ps, 0, q, kc, start=(kc == 0), stop=False)
                for q in range(4):
                    mm_quarter(pst0[q], 0, q, NKH - 2, start=False, stop=False)
                for q in range(nq1):
                    ps = psump.tile([128, BL], F32, tag="ps", name=f"ps1_{q}")
                    pst1[q] = ps
                    for kc in range(NKH - 1):
                        mm_quarter(ps, 1, q, kc, start=(kc == 0), stop=False)
                if t == 1:
                    # the last W_hh chunk's DMA is still in flight here; keep
                    # the PE p-state warm with throwaway matmuls into a scratch
                    # bank (results never read) while the kc=7 closers wait
                    warm = psump.tile([128, BL], F32, tag="ps", name="ps_warm")
                    for _ in range(14):
                        nc.tensor.matmul(
                            warm[:, :], whh_sb[:, 0:128], h_prev[:, 0:BL],
                            start=True, stop=True,
                        )
                closer_q = (0, 1, 3, 2) if t == 1 else (0, 1, 2, 3)
                for q in closer_q:
                    mm_quarter(pst0[q], 0, q, NKH - 1, start=False, stop=True)
                    if q < nq1:
                        mm_quarter(pst1[q], 1, q, NKH - 1, start=False, stop=True)
                if t == 1:
                    # hc1-q3 ran zero matmuls so far; do its full group now
                    ps = psump.tile([128, BL], F32, tag="ps", name="ps1_3")
                    pst1[3] = ps
                    for kc in range(NKH):
                        mm_quarter(ps, 1, 3, kc,
                                   start=(kc == 0), stop=(kc == NKH - 1))
                gates_and_cell(0, pst0, c_prev, c_new, h16, last=False)
                gates_and_cell(1, pst1, c_prev, c_new, h16, last=False)
                for hc in range(2, NHC):
                    last = hc == NHC - 1
                    fuse = last and t == T - 1
                    pstiles = {}
                    qseq = (0, 3, 1, 2) if last else (0, 1, 2, 3)
                    for q in qseq:
                        ps = psump.tile([128, BL], F32, tag="ps")
                        pstiles[q] = ps
                        for kc in range(NKH):
                            mm_quarter(ps, hc, q, kc,
                                       start=(kc == 0),
                                       stop=(kc == NKH - 1 and not fuse))
                    gates_and_cell(hc, pstiles, c_prev, c_new, h16, last,
                                   fuse_xg_mm=fuse)
                    if t == T - 1:
                        # last step: stream out chunks as they complete so the
                        # kernel tail isn't one big dependent DMA
                        if hc == 2:
                            nc.sync.dma_start(hs[t, :, 0:2 * BL], h16[:, 0:2 * BL])
                        if hc >= 2:
                            nc.sync.dma_start(
                                hs[t, :, hc * BL:(hc + 1) * BL],
                                h16[:, hc * BL:(hc + 1) * BL],
                            )
                if t < T - 1:
                    nc.sync.dma_start(hs[t], h16[:, :])
                h_prev, c_prev = h16, c_new

    nc.compile()
    return nc


def timeline_ns():
    from concourse.timeline_sim import TimelineSim
    nc = _get_nc()
    ts = TimelineSim(nc)
    ts.simulate()
    return ts.time


def _get_nc():
    global _cached_nc
    if _cached_nc is None:
        _cached_nc = _build()
    return _cached_nc


def kernel(x, W_ih, W_hh, b_ih, b_hh):
    global LAST_EXEC_NS, LAST_RESULTS
    nc = _get_nc()
    x = np.asarray(x, np.float32)
    W_ih = np.asarray(W_ih, np.float32)
    W_hh = np.asarray(W_hh, np.float32)
    b_ih = np.asarray(b_ih, np.float32)
    b_hh = np.asarray(b_hh, np.float32)

    # [kc][p][gi][hc][c] -> quarter order (i,f,o,g) -> [hc][kc][p][q*128+c]
    wih_pack = np.ascontiguousarray(
        W_ih.T.reshape(NKX, 128, 4, NKH, 128)[:, :, QGATE, :, :]
        .transpose(3, 0, 1, 2, 4)
        .reshape(NHC, NKX, 128, 512)
    ).astype(np.float16)
    whh_pack = np.ascontiguousarray(W_hh.T.reshape(NKH, 128, G4)).astype(np.float16)
    bg = (b_ih + b_hh).astype(np.float32)
    biasg_pack = np.ascontiguousarray(bg.reshape(NM, 128).T).astype(np.float32)
    bias_pack = np.ascontiguousarray(
        (INIT * W_hh.sum(1) + bg).reshape(NM, 128).T
    ).astype(np.float32)

    in_maps = []
    for c in range(NCORES):
        xa = np.ascontiguousarray(x[c * BL:(c + 1) * BL].T)
        in_maps.append({
            "wih": wih_pack,
            "whh": whh_pack,
            "xp": xa.reshape(NKX, 128, BL).astype(np.float16),
            "bias0": bias_pack,
            "biasg": biasg_pack,
        })

    trace = os.environ.get("LSTM_TRACE") == "1"
    res = run_bass_kernel_spmd(
        nc, in_maps, core_ids=list(range(NCORES)), trace=trace
    )
    LAST_EXEC_NS = res.exec_time_ns
    LAST_RESULTS = res

    out = np.empty((T, B, H), np.float32)
    for c in range(NCORES):
        a = np.asarray(res.results[c]["hs"], np.float32).reshape(T, 128, NKH, BL)
        out[:, c * BL:(c + 1) * BL, :] = a.transpose(0, 3, 2, 1).reshape(T, BL, H)
    return out
